# revision 13
# baseline (speedup 1.0000x reference)
"""Trainium2 Bass kernel for nn_Attention_85813446574600.

Reference computes:
    s_x = x @ W[:F] + b            # [B,T,1]
    s_c = context @ W[F:]          # [C,1]
    scores = s_x + s_c             # [B,T,C,1]
    att = softmax(scores, axis=-1) # softmax over a SIZE-1 axis -> exactly 1.0
    out = einsum('btc,btf->bcf', att, x)

Since softmax over the last (size-1) axis is identically 1.0 for any finite
scores, the output is exactly out[b,c,f] = sum_t x[b,t,f], independent of c
(and of context/W/b entirely).

V6 design (per core, batch-sharded 32/8 = 4 batches):

  sync (HWDGE)  : loads the all-ones tile (NEFF Const) plus each batch as a
                  [128, 4F] fp32 tile (partition p holds 4 consecutive T
                  rows, contiguous 8KB descriptors); then writes each
                  [256, F] output slab with a single DMA whose source AP
                  reads the [128, F] result twice (free-dim broadcast).
  vector        : one wide add per batch folds 4 T-rows to 2 while casting
                  fp32 -> bf16; then copies each finished PSUM bank to SBUF
                  (bf16).
  tensor        : two single-pass bf16 matmuls per batch against the
                  all-ones [128,128] stationary tile accumulate into one
                  PSUM bank -- summing the remaining row pair via PSUM
                  accumulation, summing across the 128 partitions, and
                  broadcasting to all 128 output partitions.

  The output DRAM tensor is bf16 (halves the store traffic; the T-sums are
  ~1e-3 relative accurate in bf16, versus the 2e-2 gate); kernel() upcasts
  to fp32 on the host. Unused DMA queue-sets are declared with num_queues=1
  and the block-exit all-engine barrier is elided: every cross-engine
  dependency is explicitly semaphore-gated, so idle engines retire early
  instead of waiting for the last output DMA.
"""

import sys

for _p in ("/opt/trn_rl_repo",):
    if _p not in sys.path:
        sys.path.insert(0, _p)

from contextlib import ExitStack

import numpy as np

import concourse.bass as bass
import concourse.mybir as mybir
from concourse.bass_utils import run_bass_kernel_spmd

# Problem shapes (hardcoded per harness contract)
B, T, C, F = 32, 512, 256, 512
N_CORES = 8
B_LOC = B // N_CORES  # 4 batches per core
P = 128               # SBUF/PSUM partitions
L = T // P            # 4 T-rows folded into each partition
DT = mybir.dt.float32
BF = mybir.dt.bfloat16

_NC_CACHE = {}


def _build_nc():
    # Skip the framework all-engine barriers (Bass.__init__ and block exit)
    # and the const-AP memsets: nothing here reads the const APs, and every
    # cross-engine dependency is explicitly semaphore-gated. The Pool
    # memsets would otherwise be the first instruction of the profile's
    # measured window, and the block-exit barrier would keep every engine
    # alive until the last output DMA lands.
    _orig_barrier = bass.Bass.all_engine_barrier
    bass.Bass.all_engine_barrier = lambda self, sem_only=False: None
    bass.BassGpSimd.memset = lambda self, ap, constant: None
    try:
        nc = bass.Bass("TRN2", target_bir_lowering=False)

        # Unused DMA queue-sets get a single ring instead of 16.
        for q in nc.m.queues:
            if q.name in ("qPoolDynamic", "qActDynamicHW"):
                q.num_queues = 1

        x = nc.dram_tensor("x", [B_LOC, T, F], DT, kind="ExternalInput").ap()
        out = nc.dram_tensor("out", [B_LOC, C, F], BF, kind="ExternalOutput").ap()

        import ml_dtypes

        ones_dram = nc.inline_tensor(
            np.ones((P, P), dtype=ml_dtypes.bfloat16), name="ones_const"
        ).ap()

        with ExitStack() as ctx:
            ec = ctx.enter_context
            ones = ec(nc.sbuf_tensor("ones", [P, P], BF)).ap()
            xst = [
                ec(nc.sbuf_tensor(f"xst{b}", [P, L * F], DT)).ap()
                for b in range(B_LOC)
            ]
            pairs = [
                ec(nc.sbuf_tensor(f"pair{b}", [P, 2 * F], BF)).ap()
                for b in range(B_LOC)
            ]
            ots = [
                ec(nc.sbuf_tensor(f"ot{b}", [P, 2 * F], BF)).ap()
                for b in range(B_LOC)
            ]
            accs = [
                ec(nc.psum_tensor(f"acc{b}", [P, F], DT)).ap() for b in range(B_LOC)
            ]

            in_sem = ec(nc.semaphore("in_sem"))
            dve_sem = ec(nc.semaphore("dve_sem"))  # +1 per DVE op, program order
            act_sem = ec(nc.semaphore("act_sem"))  # +1 per ACT copy
            pe_sem = ec(nc.semaphore("pe_sem"))    # +1 per finished batch matmul
            osem = ec(nc.semaphore("osem"))

            block = ec(nc.Block(no_gpsimd_drain=True))

            ALL_IN = 16 * (B_LOC + 1)
            # DVE program order: add1(b0)=1, add1(b1)=2, cp0=3, add1(b2)=4,
            # cp1=5, add1(b3)=6, cp2=7, cp3=8
            add_done = {0: 1, 1: 2, 2: 4, 3: 6}
            cp_done = {0: 3, 1: 5, 2: 7, 3: 8}

            @block.sync
            def _(sync):
                sync.dma_start(ones, ones_dram).then_inc(in_sem, 16)
                for b in range(B_LOC):
                    src = x[b].rearrange("(p l) f -> p l f", p=P)
                    sync.dma_start(
                        xst[b].rearrange("p (l f) -> p l f", l=L), src
                    ).then_inc(in_sem, 16)
                for b in range(B_LOC):
                    sync.wait_ge(dve_sem, cp_done[b])
                    sync.wait_ge(act_sem, b + 1)
                    # one fully-linear DMA per slab: partition p writes DRAM
                    # rows 2p and 2p+1 as one contiguous 2KB chunk
                    sync.dma_start(
                        out[b].rearrange("(p h) f -> p (h f)", h=2), ots[b]
                    ).then_inc(osem, 16)
                sync.wait_ge(osem, 16 * B_LOC)

            @block.vector
            def _(vector):
                vector.wait_ge(in_sem, ALL_IN)

                def add1(b):
                    # fold 4 T-rows to 2, casting fp32 -> bf16 on the way out
                    nc.vector.tensor_add(
                        pairs[b], xst[b][:, 0 : 2 * F], xst[b][:, 2 * F : 4 * F]
                    ).then_inc(dve_sem, 1)

                def cp(b):
                    vector.wait_ge(pe_sem, b + 1)
                    nc.vector.tensor_copy(ots[b][:, 0:F], accs[b]).then_inc(
                        dve_sem, 1
                    )

                add1(0)
                add1(1)
                cp(0)
                add1(2)
                cp(1)
                add1(3)
                cp(2)
                cp(3)

            @block.scalar
            def _(scalar):
                for b in range(B_LOC):
                    scalar.wait_ge(dve_sem, cp_done[b])
                    nc.scalar.copy(
                        ots[b][:, F : 2 * F], ots[b][:, 0:F]
                    ).then_inc(act_sem, 1)

            @block.tensor
            def _(tensor):
                tensor.wait_ge(in_sem, 16)  # ones tile
                for b in range(B_LOC):
                    tensor.wait_ge(dve_sem, add_done[b])
                    nc.tensor.matmul(
                        accs[b], ones, pairs[b][:, 0:F], start=True, stop=False
                    )
                    nc.tensor.matmul(
                        accs[b], ones, pairs[b][:, F : 2 * F], start=False, stop=True
                    ).then_inc(pe_sem, 1)

    finally:
        bass.Bass.all_engine_barrier = _orig_barrier
        del bass.BassGpSimd.memset

    return nc


def _get_nc():
    if "nc" not in _NC_CACHE:
        _NC_CACHE["nc"] = _build_nc()
    return _NC_CACHE["nc"]


def kernel(x, context=None, W=None, b=None, **_unused):
    """Full inputs in, full output out. context/W/b provably do not affect
    the output (softmax over a size-1 axis is identically 1)."""
    x = np.ascontiguousarray(np.asarray(x), dtype=np.float32)
    assert x.shape == (B, T, F), x.shape

    nc = _get_nc()
    in_maps = [{"x": x[i * B_LOC : (i + 1) * B_LOC]} for i in range(N_CORES)]
    res = run_bass_kernel_spmd(nc, in_maps, core_ids=list(range(N_CORES)))
    return np.concatenate(
        [np.asarray(r["out"], dtype=np.float32) for r in res.results], axis=0
    )


# revision 16
# speedup vs baseline: 1.1030x; 1.1030x over previous
"""Trainium2 Bass kernel for nn_Attention_85813446574600.

Reference computes:
    s_x = x @ W[:F] + b            # [B,T,1]
    s_c = context @ W[F:]          # [C,1]
    scores = s_x + s_c             # [B,T,C,1]
    att = softmax(scores, axis=-1) # softmax over a SIZE-1 axis -> exactly 1.0
    out = einsum('btc,btf->bcf', att, x)

Since softmax over the last (size-1) axis is identically 1.0 for any finite
scores, the output is exactly out[b,c,f] = sum_t x[b,t,f], independent of c
(and of context/W/b entirely).

V6 design (per core, batch-sharded 32/8 = 4 batches):

  sync (HWDGE)  : loads the all-ones tile (NEFF Const) plus each batch as a
                  [128, 4F] fp32 tile (partition p holds 4 consecutive T
                  rows, contiguous 8KB descriptors); then writes each
                  [256, F] output slab with a single DMA whose source AP
                  reads the [128, F] result twice (free-dim broadcast).
  vector        : one wide add per batch folds 4 T-rows to 2 while casting
                  fp32 -> bf16; then copies each finished PSUM bank to SBUF
                  (bf16).
  tensor        : two single-pass bf16 matmuls per batch against the
                  all-ones [128,128] stationary tile accumulate into one
                  PSUM bank -- summing the remaining row pair via PSUM
                  accumulation, summing across the 128 partitions, and
                  broadcasting to all 128 output partitions.

  The output DRAM tensor is bf16 (halves the store traffic; the T-sums are
  ~1e-3 relative accurate in bf16, versus the 2e-2 gate); kernel() upcasts
  to fp32 on the host. Unused DMA queue-sets are declared with num_queues=1
  and the block-exit all-engine barrier is elided: every cross-engine
  dependency is explicitly semaphore-gated, so idle engines retire early
  instead of waiting for the last output DMA.
"""

import sys

for _p in ("/opt/trn_rl_repo",):
    if _p not in sys.path:
        sys.path.insert(0, _p)

from contextlib import ExitStack

import numpy as np

import concourse.bass as bass
import concourse.mybir as mybir
from concourse.bass_utils import run_bass_kernel_spmd

# Problem shapes (hardcoded per harness contract)
B, T, C, F = 32, 512, 256, 512
N_CORES = 8
B_LOC = B // N_CORES  # 4 batches per core
P = 128               # SBUF/PSUM partitions
L = T // P            # 4 T-rows folded into each partition
DT = mybir.dt.float32
BF = mybir.dt.bfloat16

_NC_CACHE = {}


def _build_nc():
    # Skip the framework all-engine barriers (Bass.__init__ and block exit)
    # and the const-AP memsets: nothing here reads the const APs, and every
    # cross-engine dependency is explicitly semaphore-gated. The Pool
    # memsets would otherwise be the first instruction of the profile's
    # measured window, and the block-exit barrier would keep every engine
    # alive until the last output DMA lands.
    _orig_barrier = bass.Bass.all_engine_barrier
    bass.Bass.all_engine_barrier = lambda self, sem_only=False: None
    bass.BassGpSimd.memset = lambda self, ap, constant: None
    try:
        nc = bass.Bass("TRN2", target_bir_lowering=False, monotonic_sem_count=0)

        # Unused DMA queue-sets get a single ring instead of 16.
        for q in nc.m.queues:
            if q.name in ("qPoolDynamic", "qActDynamicHW"):
                q.num_queues = 1

        x = nc.dram_tensor("x", [B_LOC, T, F], DT, kind="ExternalInput").ap()
        out = nc.dram_tensor("out", [B_LOC, C, F], BF, kind="ExternalOutput").ap()

        import ml_dtypes

        ones_dram = nc.inline_tensor(
            np.ones((P, P), dtype=ml_dtypes.bfloat16), name="ones_const"
        ).ap()

        with ExitStack() as ctx:
            ec = ctx.enter_context
            ones = ec(nc.sbuf_tensor("ones", [P, P], BF)).ap()
            xst = [
                ec(nc.sbuf_tensor(f"xst{b}", [P, L * F], DT)).ap()
                for b in range(B_LOC)
            ]
            pairs = [
                ec(nc.sbuf_tensor(f"pair{b}", [P, 2 * F], BF)).ap()
                for b in range(B_LOC)
            ]
            ots = [
                ec(nc.sbuf_tensor(f"ot{b}", [P, 2 * F], BF)).ap()
                for b in range(B_LOC)
            ]
            accs = [
                ec(nc.psum_tensor(f"acc{b}", [P, F], DT)).ap() for b in range(B_LOC)
            ]

            in_sem = ec(nc.semaphore("in_sem"))
            dve_sem = ec(nc.semaphore("dve_sem"))  # +1 per DVE op, program order
            act_sem = ec(nc.semaphore("act_sem"))  # +1 per ACT copy
            pe_sem = ec(nc.semaphore("pe_sem"))    # +1 per finished batch matmul
            osem = ec(nc.semaphore("osem"))

            block = ec(nc.Block(no_gpsimd_drain=True))

            ALL_IN = 16 * (B_LOC + 1)
            # DVE program order: add1(b0)=1, add1(b1)=2, cp0=3, add1(b2)=4,
            # cp1=5, add1(b3)=6, cp2=7, cp3=8
            add_done = {0: 1, 1: 2, 2: 4, 3: 6}
            cp_done = {0: 3, 1: 5, 2: 7, 3: 8}

            @block.sync
            def _(sync):
                sync.dma_start(ones, ones_dram).then_inc(in_sem, 16)
                for b in range(B_LOC):
                    src = x[b].rearrange("(p l) f -> p l f", p=P)
                    sync.dma_start(
                        xst[b].rearrange("p (l f) -> p l f", l=L), src
                    ).then_inc(in_sem, 16)
                for b in range(B_LOC):
                    sync.wait_ge(dve_sem, cp_done[b])
                    sync.wait_ge(act_sem, b + 1)
                    # one fully-linear DMA per slab: partition p writes DRAM
                    # rows 2p and 2p+1 as one contiguous 2KB chunk
                    sync.dma_start(
                        out[b].rearrange("(p h) f -> p (h f)", h=2), ots[b]
                    ).then_inc(osem, 16)
                # No explicit osem wait: the framework epilogue's semaphore
                # final-value checks already gate NEFF completion on osem=64,
                # and skipping it releases the end-of-function gather barrier
                # (and with it the slow PE-sequencer teardown) ~2us earlier.

            @block.vector
            def _(vector):
                vector.wait_ge(in_sem, ALL_IN)

                def add1(b):
                    # fold 4 T-rows to 2, casting fp32 -> bf16 on the way out
                    nc.vector.tensor_add(
                        pairs[b], xst[b][:, 0 : 2 * F], xst[b][:, 2 * F : 4 * F]
                    ).then_inc(dve_sem, 1)

                def cp(b):
                    vector.wait_ge(pe_sem, b + 1)
                    nc.vector.tensor_copy(ots[b][:, 0:F], accs[b]).then_inc(
                        dve_sem, 1
                    )

                add1(0)
                add1(1)
                cp(0)
                add1(2)
                cp(1)
                add1(3)
                cp(2)
                cp(3)

            @block.scalar
            def _(scalar):
                # dummy first activation: pulls the one-time ACT_TABLE_LOAD
                # (~1.3us) off the copy critical path, overlapping the DVE adds
                scalar.wait_ge(in_sem, ALL_IN)
                nc.scalar.copy(ots[0][:, F : F + 1], ones[:, 0:1])
                for b in range(B_LOC):
                    scalar.wait_ge(dve_sem, cp_done[b])
                    nc.scalar.copy(
                        ots[b][:, F : 2 * F], ots[b][:, 0:F]
                    ).then_inc(act_sem, 1)

            @block.tensor
            def _(tensor):
                tensor.wait_ge(in_sem, 16)  # ones tile
                for b in range(B_LOC):
                    tensor.wait_ge(dve_sem, add_done[b])
                    nc.tensor.matmul(
                        accs[b], ones, pairs[b][:, 0:F], start=True, stop=False
                    )
                    nc.tensor.matmul(
                        accs[b], ones, pairs[b][:, F : 2 * F], start=False, stop=True
                    ).then_inc(pe_sem, 1)

    finally:
        bass.Bass.all_engine_barrier = _orig_barrier
        del bass.BassGpSimd.memset

    return nc


def _get_nc():
    if "nc" not in _NC_CACHE:
        _NC_CACHE["nc"] = _build_nc()
    return _NC_CACHE["nc"]


def kernel(x, context=None, W=None, b=None, **_unused):
    """Full inputs in, full output out. context/W/b provably do not affect
    the output (softmax over a size-1 axis is identically 1)."""
    x = np.ascontiguousarray(np.asarray(x), dtype=np.float32)
    assert x.shape == (B, T, F), x.shape

    nc = _get_nc()
    in_maps = [{"x": x[i * B_LOC : (i + 1) * B_LOC]} for i in range(N_CORES)]
    res = run_bass_kernel_spmd(nc, in_maps, core_ids=list(range(N_CORES)))
    return np.concatenate(
        [np.asarray(r["out"], dtype=np.float32) for r in res.results], axis=0
    )
